# revision 13
# baseline (speedup 1.0000x reference)
"""Trainium2 Bass kernel for nn_AttentionLateralOp (lateral self-attention).

Reference computation (B=4, C=512, H=W=64, N=H*W=4096, CQ=C//8=64):
    f  = Wq @ x_t            # [B, CQ, N]   query from target
    g  = Wk @ x_o            # [B, CQ, N]   key from origin
    hh = Wv @ x_o            # [B, C,  N]   value from origin
    scores[m, n] = sum_q f[q, m] * g[q, n]          # [B, N, N]
    beta = softmax(scores, axis=m)
    out[c, n] = gamma * sum_m hh[c, m] * beta[m, n] + x_t[c, n]

Sharding: 8 cores = (batch b in 0..3) x (half of the n axis). Each core holds
full f/hh for its batch (softmax is over the full m axis) and a 2048-wide
slice of g / x_t / output. No collectives.

Key speed trick: the two big attention matmuls (o = hh @ E and the column-sum
ones @ E) run in fp8 with MatmulPerfMode.DoubleRow (256-wide contraction,
2x PE throughput). Making E fp8-representable needs a per-column shift of the
scores before exp; that shift is folded into the scores matmul for free by
padding the CQ=64 contraction to 65 rows: f gets a constant 1.0 row, g gets a
per-column bias row b[n] = -submax[n], where submax is an approximate column
max from a cheap transposed pre-pass over a 256-sample subset of m. A DVE
clamp (min(shifted, 6.5)) before the exp makes fp8 overflow impossible for
any input (exp output <= e^2.5), and the top entry is always >= e^-4 so the
softmax denominator can never be all-zero. The arbitrary per-column scale
cancels exactly between numerator and denominator. gamma is folded into Wv
on the host (rescaled by a power of two alpha if needed; alpha is undone in
the epilogue).

Per-core pipeline (all per rep):
    f = Wq @ x_t (bf16)            -> f_sb [128, 4096] (row 64 = 1.0)
    g = Wk @ x_o_slice (bf16)      -> g_sb [128, 2048] (row 64 = bias, DMA'd)
    pre-pass: st_sub[n, m_sub] = g^T f_sub; DVE free-axis max -> bias row
    hh^T (fp8 DoubleRow from host-converted x_o, gamma*Wv) -> hh_sb fp8
    per 512-wide n-chunk, software-pipelined (LAG=2):
        scores matmul (bf16, shift folded in) -> PSUM
        DVE clamp -> fp16, ACT exp(x - 4) -> E' fp8e4 pair tiles [128,2,512]
        s2 += ones8^T @ E'   (fp8 DoubleRow)
        o[ci] += hh^T[ci] @ E'  (fp8 DoubleRow, 4 c-tiles)
      epilogue: recip(s2) * alpha, out = o * recip + x_t (fp32), DMA out
"""

import os
import threading

import numpy as np
import ml_dtypes

import concourse.bass as bass
import concourse.tile as tile
from concourse import bacc, mybir
from concourse.bass_utils import run_bass_kernel_spmd

B = 4
C = 512
HW = 64
N = HW * HW          # 4096
CQ = 64              # query/key channels
P = 128              # partitions
CT = C // P          # 4  c-tiles
MT = N // P          # 32 m-tiles
NCORES = 8
NSL = N // (NCORES // B)      # 2048: n-slice per core
NCH = 512                     # n-chunk (one PSUM bank of fp32)
NCHUNKS = NSL // NCH          # 4
NPAIR = MT // 2               # 16 m-tile pairs per chunk
NT = NSL // P                 # 16 n-tiles of 128 (pre-pass)
LAG = 2                       # software pipeline depth (pairs)
CAP = 14.0                    # clamp on shifted scores (pre exp-bias)
EXPB = -4.0                   # exp bias; E' <= e^{CAP+EXPB} = e^10

F32 = mybir.dt.float32
FP16 = mybir.dt.float16
BF16 = mybir.dt.bfloat16
FP8E4 = mybir.dt.float8e4
FP8E5 = mybir.dt.float8e5
DR = mybir.MatmulPerfMode.DoubleRow


def _build_bass(phase=None, reps=1):
    nc = bacc.Bacc(trn_type="TRN2")

    xt_full = nc.dram_tensor("xt_full", [C, N], BF16, kind="ExternalInput")
    xo_sl = nc.dram_tensor("xo_sl", [C, NSL], BF16, kind="ExternalInput")
    xo8 = nc.dram_tensor("xo8", [C, N], FP8E4, kind="ExternalInput")
    xt_sl = nc.dram_tensor("xt_sl", [C, NSL], F32, kind="ExternalInput")
    wq_t = nc.dram_tensor("wq_t", [C, CQ], BF16, kind="ExternalInput")
    wk_t = nc.dram_tensor("wk_t", [C, CQ], BF16, kind="ExternalInput")
    wv8 = nc.dram_tensor("wv8", [C, C], FP8E4, kind="ExternalInput")
    alpha = nc.dram_tensor("alpha", [1, 1], F32, kind="ExternalInput")
    ident = nc.dram_tensor("ident", [P, P], BF16, kind="ExternalInput")
    out = nc.dram_tensor("out", [C, NSL], F32, kind="ExternalOutput")
    dump = os.environ.get("KDUMP") == "1"
    if dump:
        f_dump = nc.dram_tensor("f_dump", [P, N], BF16, kind="ExternalOutput")
        g_dump = nc.dram_tensor("g_dump", [P, NSL], BF16, kind="ExternalOutput")
        hh_dump = nc.dram_tensor("hh_dump", [P, MT, C], FP8E4, kind="ExternalOutput")
        e_dump = nc.dram_tensor("e_dump", [P, 2, NCH], FP8E5, kind="ExternalOutput")
        sc_dump = nc.dram_tensor("sc_dump", [P, 2, NCH], FP16, kind="ExternalOutput")
        s2_dump = nc.dram_tensor("s2_dump", [P, NCH], F32, kind="ExternalOutput")
        bn_dump = nc.dram_tensor("bn_dump", [P, NT], BF16, kind="ExternalOutput")
        sm_dump = nc.dram_tensor("sm_dump", [P, NT], F32, kind="ExternalOutput")

    with tile.TileContext(nc) as tc:
        with (
            tc.tile_pool(name="const", bufs=1) as const,
            tc.tile_pool(name="sc16p", bufs=3) as sc16p,
            tc.tile_pool(name="e8p", bufs=4) as e8p,
            tc.tile_pool(name="work", bufs=4) as work,
            tc.tile_pool(name="ps_sc", bufs=2, space="PSUM") as ps_sc,
            tc.tile_pool(name="ps_bt", bufs=1, space="PSUM") as ps_bt,
            tc.tile_pool(name="ps_s2", bufs=1, space="PSUM") as ps_s2,
            tc.tile_pool(name="ps_o", bufs=4, space="PSUM") as ps_o,
        ):
            # ---- weights / constants ----
            wq_sb = const.tile([P, CT, CQ], BF16)
            nc.sync.dma_start(wq_sb, wq_t.rearrange("(ct p) q -> p ct q", p=P))
            wk_sb = const.tile([P, CT, CQ], BF16)
            nc.sync.dma_start(wk_sb, wk_t.rearrange("(ct p) q -> p ct q", p=P))
            wv8_sb = const.tile([P, CT, C], FP8E4)
            nc.sync.dma_start(wv8_sb, wv8.rearrange("(ct p) c -> p ct c", p=P))
            alpha_sb = const.tile([P, 1], F32)
            nc.gpsimd.dma_start(out=alpha_sb, in_=alpha[:, :].to_broadcast([P, 1]))
            ones8_sb = const.tile([P, 2, P], FP8E5)
            nc.vector.memset(ones8_sb, 1.0)
            expbias_sb = const.tile([P, 1], F32)
            nc.vector.memset(expbias_sb, EXPB)
            capb_sb = const.tile([P, 1], F32)
            nc.vector.memset(capb_sb, CAP + EXPB)
            cap_sb = const.tile([P, 1], F32)
            nc.vector.memset(cap_sb, CAP)
            ident_sb = const.tile([P, P], BF16)
            nc.sync.dma_start(ident_sb, ident[:, :])

            # ---- stream in inputs ----
            xt_tiles = []
            for ci in range(CT):
                t = const.tile([P, N], BF16, name=f"xt_{ci}")
                nc.sync.dma_start(t, xt_full[ci * P:(ci + 1) * P, :])
                xt_tiles.append(t)
            xo_sl_sb = const.tile([P, CT, NSL], BF16)
            nc.sync.dma_start(xo_sl_sb, xo_sl.rearrange("(ct p) n -> p ct n", p=P))
            xo8_sb = const.tile([P, CT, N], FP8E4)
            nc.sync.dma_start(xo8_sb, xo8.rearrange("(ct p) n -> p ct n", p=P))
            xt_sl_sb = const.tile([P, CT, NSL], F32)
            for ci in range(CT):
                nc.sync.dma_start(xt_sl_sb[:, ci, :], xt_sl[ci * P:(ci + 1) * P, :])

            f_sb = const.tile([P, N], BF16)
            nc.vector.memset(f_sb[CQ:P, :], 0.0)
            nc.vector.memset(f_sb[CQ:CQ + 1, :], 1.0)     # ones row (bias matmul)
            g_sb = const.tile([P, NSL], BF16)
            nc.vector.memset(g_sb[CQ:P, :], 0.0)
            hh_sb = const.tile([P, MT, C], FP8E4)
            submax_nt = const.tile([P, NT], F32)
            bias_nt = const.tile([P, NT], BF16)
            bias_tr = const.tile([NT, P], BF16)

            # f subsample for the pre-pass: m-tiles 8 and 24 (stride 16*P)
            f_sub = f_sb.rearrange("p (mt q) -> p mt q", q=P)[:, 4:29:8, :]

            for _rep in range(reps):
                # zero the g bias row so the pre-pass contraction is clean
                nc.gpsimd.memset(g_sb[CQ:CQ + 1, :], 0.0)

                # ---- f = Wq @ x_t ----
                for mc in range(N // NCH):
                    ps = ps_sc.tile([P, NCH], F32, tag="sc", name="f_ps")
                    for ci in range(CT):
                        nc.tensor.matmul(
                            ps[:CQ, :],
                            wq_sb[:, ci, :],
                            xt_tiles[ci][:, mc * NCH:(mc + 1) * NCH],
                            start=(ci == 0),
                            stop=(ci == CT - 1),
                        )
                    nc.scalar.activation(
                        f_sb[:CQ, mc * NCH:(mc + 1) * NCH], ps[:CQ, :],
                        mybir.ActivationFunctionType.Copy,
                    )

                # ---- g = Wk @ x_o_slice ----
                for gc in range(NCHUNKS):
                    ps = ps_sc.tile([P, NCH], F32, tag="sc", name="g_ps")
                    for ci in range(CT):
                        nc.tensor.matmul(
                            ps[:CQ, :],
                            wk_sb[:, ci, :],
                            xo_sl_sb[:, ci, gc * NCH:(gc + 1) * NCH],
                            start=(ci == 0),
                            stop=(ci == CT - 1),
                        )
                    nc.scalar.activation(
                        g_sb[:CQ, gc * NCH:(gc + 1) * NCH], ps[:CQ, :],
                        mybir.ActivationFunctionType.Copy,
                    )

                # ---- pre-pass: approx column max of scores over m subset ----
                for t in range(NT):
                    ps = ps_sc.tile([P, NCH], F32, tag="sc", name="pre_ps")
                    nc.tensor.matmul(
                        ps[:, :4 * P],
                        g_sb[:, t * P:(t + 1) * P],
                        f_sub,
                        start=True,
                        stop=True,
                    )
                    nc.vector.reduce_max(
                        submax_nt[:, t:t + 1], ps[:, :4 * P],
                        axis=mybir.AxisListType.X,
                    )
                nc.vector.tensor_scalar_mul(bias_nt, submax_nt, -1.0)
                # transpose [128, 16] -> [16, 128], then one DMA lays the
                # row out as [1, 2048] with n = t*128 + p
                btr_ps = ps_bt.tile([NT, P], BF16, tag="btr", name="btr_ps")
                nc.tensor.transpose(btr_ps, bias_nt, ident_sb)
                nc.vector.tensor_copy(out=bias_tr, in_=btr_ps)
                nc.sync.dma_start(
                    g_sb[CQ:CQ + 1, :].rearrange("o (t p) -> o t p", p=P),
                    bias_tr[:, :],
                )

                # ---- hh^T[m, c] = (gamma*Wv @ x_o)^T, fp8 DoubleRow ----
                for mi in range(MT):
                    ps = ps_sc.tile([P, C], F32, tag="sc", name="hh_ps")
                    for jj in range(2):
                        nc.tensor.matmul(
                            ps,
                            xo8_sb[:, 2 * jj:2 * jj + 2, mi * P:(mi + 1) * P],
                            wv8_sb[:, 2 * jj:2 * jj + 2, :],
                            start=(jj == 0),
                            stop=(jj == 1),
                            perf_mode=DR,
                        )
                    if mi % 2 == 0:
                        nc.scalar.activation(
                            hh_sb[:, mi, :], ps,
                            mybir.ActivationFunctionType.Copy,
                        )
                    else:
                        nc.vector.tensor_copy(out=hh_sb[:, mi, :], in_=ps)

                if dump:
                    nc.sync.dma_start(bn_dump[:, :], bias_nt)
                    nc.sync.dma_start(sm_dump[:, :], submax_nt)
                    nc.sync.dma_start(f_dump[:, :], f_sb)
                    nc.sync.dma_start(g_dump[:, :], g_sb)
                    nc.sync.dma_start(hh_dump[:, :, :], hh_sb)

                # ---- attention chunks over the local n axis ----
                for ch in range(NCHUNKS):
                    nsl = slice(ch * NCH, (ch + 1) * NCH)
                    s2_ps = ps_s2.tile([P, NCH], F32, tag="s2", name="s2_ps")
                    o_ps = [
                        ps_o.tile([P, NCH], F32, tag="o", name=f"o_ps{ci}")
                        for ci in range(CT)
                    ]
                    e8_ring = [None] * (LAG + 2)
                    for p in range(NPAIR + LAG):
                        if p < NPAIR:
                            # route ~1/3 of pairs through ACT to balance the
                            # clamp cost: min(x,CAP) = CAP - Relu(CAP - x),
                            # with the affine absorbed into the exp
                            act_route = (p % 3 == 2)
                            sc16_t = sc16p.tile([P, 2, NCH], FP16, tag="sc16",
                                                name="sc16_t")
                            e8_t = e8p.tile([P, 2, NCH], FP8E5, tag="e8",
                                            name="e8_t")
                            for j in range(2):
                                mi = 2 * p + j
                                ps = ps_sc.tile([P, NCH], F32, tag="sc",
                                                name="sc_ps")
                                nc.tensor.matmul(
                                    ps,
                                    f_sb[:, mi * P:(mi + 1) * P],
                                    g_sb[:, nsl],
                                    start=True,
                                    stop=True,
                                )
                                if act_route:
                                    nc.scalar.activation(
                                        sc16_t[:, j, :], ps,
                                        mybir.ActivationFunctionType.Relu,
                                        bias=cap_sb, scale=-1.0,
                                    )
                                else:
                                    nc.vector.tensor_scalar_min(
                                        sc16_t[:, j, :], ps, CAP
                                    )
                            if act_route:
                                # sc16 holds t = Relu(CAP+EXPB - x); undo via
                                # exp(-t + CAP+EXPB) = exp(min(x-EXPB.., ..))
                                nc.scalar.activation(
                                    e8_t, sc16_t,
                                    mybir.ActivationFunctionType.Exp,
                                    bias=capb_sb, scale=-1.0,
                                )
                            else:
                                nc.scalar.activation(
                                    e8_t, sc16_t,
                                    mybir.ActivationFunctionType.Exp,
                                    bias=expbias_sb, scale=1.0,
                                )
                            if dump and ch == 0 and p == 0:
                                nc.sync.dma_start(e_dump[:, :, :], e8_t)
                                nc.sync.dma_start(sc_dump[:, :, :], sc16_t)
                            e8_ring[p % (LAG + 2)] = e8_t
                        if p >= LAG:
                            q = p - LAG
                            e8_q = e8_ring[q % (LAG + 2)]
                            nc.tensor.matmul(
                                s2_ps, ones8_sb, e8_q,
                                start=(q == 0), stop=(q == NPAIR - 1),
                                perf_mode=DR,
                            )
                            for ci in range(CT):
                                nc.tensor.matmul(
                                    o_ps[ci],
                                    hh_sb[:, 2 * q:2 * q + 2, ci * P:(ci + 1) * P],
                                    e8_q,
                                    start=(q == 0), stop=(q == NPAIR - 1),
                                    perf_mode=DR,
                                )
                    # epilogue
                    # s2 >= e^EXPB always (top entry can't flush in e5m2),
                    # so the reciprocal is safe without a guard
                    if dump and ch == 0:
                        nc.sync.dma_start(s2_dump[:, :], s2_ps)
                    recip = work.tile([P, NCH], F32, tag="w", name="recip")
                    nc.vector.reciprocal_approx_fast(out=recip, in_=s2_ps)
                    recipa = work.tile([P, NCH], F32, tag="w", name="recipa")
                    nc.vector.tensor_scalar_mul(recipa, recip, alpha_sb)
                    for ci in range(CT):
                        o_sb = work.tile([P, NCH], F32, tag="w", name="o_sb")
                        nc.vector.tensor_mul(out=o_sb, in0=o_ps[ci], in1=recipa)
                        nc.vector.tensor_add(
                            out=o_sb, in0=o_sb, in1=xt_sl_sb[:, ci, nsl]
                        )
                        nc.sync.dma_start(out[ci * P:(ci + 1) * P, nsl], o_sb)
    nc.compile()
    return nc


_lock = threading.Lock()
_cached_nc = None


def _get_nc():
    global _cached_nc
    with _lock:
        if _cached_nc is None:
            _cached_nc = _build_bass()
        return _cached_nc


def make_in_maps(origin_out, target_in, Wq, Wk, Wv, gamma):
    x_o = np.ascontiguousarray(origin_out.reshape(B, C, N), dtype=np.float32)
    x_t = np.ascontiguousarray(target_in.reshape(B, C, N), dtype=np.float32)
    x_o_bf = x_o.astype(ml_dtypes.bfloat16)
    x_o_f8 = x_o.astype(ml_dtypes.float8_e4m3)
    x_t_bf = x_t.astype(ml_dtypes.bfloat16)
    wq_t = np.ascontiguousarray(np.asarray(Wq, dtype=np.float32).T).astype(
        ml_dtypes.bfloat16
    )
    wk_t = np.ascontiguousarray(np.asarray(Wk, dtype=np.float32).T).astype(
        ml_dtypes.bfloat16
    )
    gam = float(np.asarray(gamma, dtype=np.float32).reshape(-1)[0])
    wv_g = np.ascontiguousarray(np.asarray(Wv, dtype=np.float32).T) * gam
    m = float(np.abs(wv_g).max())
    alpha = 1.0
    while m / alpha > 224.0:
        alpha *= 2.0
    wv8 = (wv_g / alpha).astype(ml_dtypes.float8_e4m3)
    alpha_arr = np.array([[alpha]], dtype=np.float32)

    in_maps = []
    for core in range(NCORES):
        b = core // (NCORES // B)
        h = core % (NCORES // B)
        sl = slice(h * NSL, (h + 1) * NSL)
        in_maps.append(
            {
                "xt_full": x_t_bf[b],
                "xo_sl": np.ascontiguousarray(x_o_bf[b][:, sl]),
                "xo8": x_o_f8[b],
                "xt_sl": np.ascontiguousarray(x_t[b][:, sl]),
                "wq_t": wq_t,
                "wk_t": wk_t,
                "wv8": wv8,
                "alpha": alpha_arr,
                "ident": np.eye(P, dtype=ml_dtypes.bfloat16),
            }
        )
    return in_maps


def assemble_output(results):
    out = np.empty((B, C, N), dtype=np.float32)
    for core in range(NCORES):
        b = core // (NCORES // B)
        h = core % (NCORES // B)
        sl = slice(h * NSL, (h + 1) * NSL)
        out[b][:, sl] = results[core]["out"]
    return out.reshape(B, C, HW, HW)


def kernel(origin_out, target_in, Wq, Wk, Wv, gamma):
    nc = _get_nc()
    in_maps = make_in_maps(origin_out, target_in, Wq, Wk, Wv, gamma)
    res = run_bass_kernel_spmd(nc, in_maps, core_ids=list(range(NCORES)))
    return assemble_output(res.results)


if __name__ == "__main__":
    rng = np.random.default_rng(0)
    inputs = {
        "origin_out": rng.standard_normal((B, C, HW, HW), dtype=np.float32),
        "target_in": rng.standard_normal((B, C, HW, HW), dtype=np.float32),
        "Wq": (rng.standard_normal((CQ, C)) / np.sqrt(C)).astype(np.float32),
        "Wk": (rng.standard_normal((CQ, C)) / np.sqrt(C)).astype(np.float32),
        "Wv": (rng.standard_normal((C, C)) / np.sqrt(C)).astype(np.float32),
        "gamma": np.zeros((1,), dtype=np.float32),
    }
    out = kernel(**inputs)
    print("kernel output", out.shape, out.dtype, float(np.abs(out).mean()))
